# revision 2
# baseline (speedup 1.0000x reference)
"""MultiHeadSelfAttention (qk-LayerNorm variant) on 8 TRN2 NeuronCores, v2.

Problem (B=4, N=2048, C=1024, H=16, D=64, fp32):
    qkv = x @ W_qkv + b_qkv ; q,k,v = split(qkv)
    q = LN(q)*scale ; k = LN(k)          (LN over full C)
    attn = softmax(q_h @ k_h^T) per head ; o = attn @ v_h
    out = concat_heads(o) @ W_proj + b_proj

Sharding: core i handles batch b=i//2 and query-half i%2. Each core computes
K/V for the full sequence of its batch and attention/proj for its 1024 query
rows. No collectives (pairwise K/V AllGather was measured at ~85us per op —
worse than the duplicated compute). Host permutes token tiles so the core's
query half is tiles 0..7 (softmax is permutation-invariant over keys when K
and V share the ordering), so one SPMD program serves all cores.

Design (vs. fp32r baseline, which measured PE cold-throttled at 1.2 GHz with
serial 270ns LDWEIGHTS on every matmul):
  - all matmul operands bf16 (host-precast): FWL fast weight load, and the
    2e-2 rel-err gate leaves plenty of margin (measured 1.3e-2).
  - x^T / K^T / Q^T / V / O^T all SBUF-resident, no DRAM staging round trip.
  - LN via bn_stats/bn_aggr; apply on ACT (Identity, per-partition
    scale/bias APs); transpose on PE; evacuation via nc.any (idle engine).
  - P2 runs on a shared 4-slot ring of [128,512] S^T-chunk psums: a chunk's
    slot frees as soon as its exp is done, before the PE needs it for the
    next kt, so the PE never stalls on the softmax (HAM stays warm).
  - exp split between ScalarE (spline exp) and VectorE (Schraudolph int16
    bit-trick, ~3.3% rel err — the softmax ratio cancels most of it).
  - the two K=64 S^T matmuls of a head pair are emitted adjacently on
    complementary PE row-groups (base partitions 0/64) to run concurrently.
  - softmax denominators via the ones-column in V (M=65 PV matmuls);
    1/den = exp(-ln(den)) on ACT, deferred into the next pair's slack;
    broadcast on GpSimd; numerators evacuated to SBUF right after PV to
    free the po banks for the next pair.
"""
import numpy as np
from contextlib import ExitStack

import concourse.bass as bass
from concourse import bacc
import concourse.tile as tile
import concourse.mybir as mybir
from concourse.masks import make_identity

dt = mybir.dt
AF = mybir.ActivationFunctionType
OP = mybir.AluOpType
ts = bass.ts

B, N, C = 4, 2048, 1024
H, D = 16, 64
NQ = 1024            # query rows per core
SCALE = D ** -0.5
EPS = 1e-6
TT = N // 128        # 16 token tiles (full seq)
TQ = NQ // 128       # 8 token tiles (query half)
CT = C // 128        # 8 channel tiles (= head pairs)
BF = dt.bfloat16

# Schraudolph exp for bf16: bits_i16 = round(x * 128/ln2 + (127*128 - CEXP))
SCHR_A = 128.0 / float(np.log(2.0))
SCHR_B = 127.0 * 128.0 - 5.0


def build_nc(reps=1, with_bias=False, with_gamma=False, act_ln_apply=True):
    nc = bacc.Bacc()
    xTt = nc.dram_tensor("xTt", [TT, 128, C], BF, kind="ExternalInput")
    wqkv = nc.dram_tensor("wqkv", [C, 3 * C], BF, kind="ExternalInput")
    wproj = nc.dram_tensor("wproj", [C, C], BF, kind="ExternalInput")
    bqkv = nc.dram_tensor("bqkv", [3 * C], BF, kind="ExternalInput")
    bproj = nc.dram_tensor("bproj", [C], BF, kind="ExternalInput")
    gq = nc.dram_tensor("gq", [C], dt.float32, kind="ExternalInput")
    bq = nc.dram_tensor("bq", [C], dt.float32, kind="ExternalInput")
    gk = nc.dram_tensor("gk", [C], dt.float32, kind="ExternalInput")
    bk = nc.dram_tensor("bk", [C], dt.float32, kind="ExternalInput")
    out = nc.dram_tensor("out", [NQ, C], dt.float32, kind="ExternalOutput")

    with tile.TileContext(nc) as tc, ExitStack() as top:
        const = top.enter_context(tc.tile_pool(name="const", bufs=1))
        res = top.enter_context(tc.tile_pool(name="res", bufs=1))

        # ---- constants ----
        ident = const.tile([128, 128], BF)
        make_identity(nc, ident[:])
        ones1 = const.tile([1, 128], BF)
        nc.vector.memset(ones1[:], 1.0)
        ones_ct = const.tile([128, CT], BF)
        nc.vector.memset(ones_ct[:], 1.0)
        eps_t = const.tile([128, 1], dt.float32)
        nc.vector.memset(eps_t[:], EPS)
        gq_t = const.tile([128, CT], dt.float32)
        bq_t = const.tile([128, CT], dt.float32)
        gk_t = const.tile([128, CT], dt.float32)
        bk_t = const.tile([128, CT], dt.float32)
        for t_, d_ in ((gq_t, gq), (bq_t, bq), (gk_t, gk), (bk_t, bk)):
            nc.sync.dma_start(t_[:], d_.rearrange("(ct p) -> p ct", p=128))
        if with_bias:
            bqkv_t = const.tile([1, 3 * C], BF)
            nc.sync.dma_start(bqkv_t[:], bqkv.rearrange("(o n) -> o n", o=1))
            bproj_t = const.tile([1, C], BF)
            nc.sync.dma_start(bproj_t[:], bproj.rearrange("(o n) -> o n", o=1))

        # ---- resident tensors (all bf16) ----
        x_sb = res.tile([128, TT, C], BF)       # x^T: [ctr-chan, tt, kt*128+tok]
        qnT = res.tile([128, CT, NQ], BF)       # Q^T LN'd  [c, t]
        knT = res.tile([128, CT, N], BF)        # K^T LN'd  [c, t]
        vres = res.tile([128, TT, CT, 130], BF)  # [V_h0 | 1 | V_h1 | 1] per kt
        oT = res.tile([128, CT, NQ], BF)        # O^T normalized  [c, t]
        wp = res.tile([128, CT, C], BF)         # W_proj (prefetched early)

        # first x chunk only; the rest stream in after the K-group weights
        xr = xTt.rearrange("t p c -> p t c")
        nc.sync.dma_start(x_sb[:, ts(0, TT // 4), :], xr[:, ts(0, TT // 4), :])
        # ones columns for the softmax row-sum trick
        nc.vector.memset(vres[:, :, :, 64], 1.0)
        nc.vector.memset(vres[:, :, :, 129], 1.0)

        for _rep in range(reps):
            # ============ P1: QKV + LN (+transpose for K/Q) ============
            with ExitStack() as p1:
                wq_p = p1.enter_context(tc.tile_pool(name="wq", bufs=2))
                ln_p = p1.enter_context(tc.tile_pool(name="ln", bufs=3))
                ps_p = p1.enter_context(tc.tile_pool(name="ps1", bufs=3, space="PSUM"))
                pst_p = p1.enter_context(tc.tile_pool(name="pst", bufs=2, space="PSUM"))

                def load_w_group(oc_base):
                    w_t = wq_p.tile([128, CT, C], BF, tag="w_t")
                    wr = wqkv.rearrange("(kt p) c -> p kt c", p=128)
                    nc.sync.dma_start(w_t[:], wr[:, :, oc_base:oc_base + C])
                    return w_t

                def qkv_psum(ps, tt, w_t, oc_base):
                    """ps [128,1024] = (x_tile.T @ Wgroup) (+ bias)."""
                    for kt in range(CT):
                        for ch in range(2):
                            nc.tensor.matmul(
                                ps[:, ts(ch, 512)],
                                x_sb[:, tt, ts(kt, 128)],
                                w_t[:, kt, ts(ch, 512)],
                                start=(kt == 0),
                                stop=(not with_bias and kt == CT - 1),
                                skip_group_check=True)
                    if with_bias:
                        for ch in range(2):
                            lo = oc_base + ch * 512
                            nc.tensor.matmul(
                                ps[:, ts(ch, 512)], ones1[:],
                                bqkv_t[:, lo:lo + 512],
                                start=False, stop=True, skip_group_check=True)

                def ln_apply_transpose(ps_tok, sink):
                    """LN stats via bn_stats, apply -> bf16 tok, PE-transpose
                    128x128 blocks, evacuate via sink(ct, psum_block)."""
                    st6 = ln_p.tile([128, 2, 6], dt.float32, tag="st6")
                    for sub in range(2):
                        nc.vector.bn_stats(
                            st6[:, sub, :],
                            ps_tok[:].rearrange("p (s f) -> p s f", s=2)[:, sub, :])
                    mv = ln_p.tile([128, 2], dt.float32, tag="mv")
                    nc.vector.bn_aggr(mv[:], st6[:])
                    sv = ln_p.tile([128, 1], dt.float32, tag="sv")
                    nc.scalar.activation(sv[:], mv[:, 1:2], AF.Sqrt, bias=eps_t[:])
                    rstd = ln_p.tile([128, 1], dt.float32, tag="rstd")
                    with nc.allow_low_precision(reason="layernorm rstd"):
                        nc.vector.reciprocal(rstd[:], sv[:])
                    nmr = ln_p.tile([128, 1], dt.float32, tag="nmr")
                    nc.vector.tensor_scalar(nmr[:], mv[:, 0:1], rstd[:], -1.0,
                                            op0=OP.mult, op1=OP.mult)
                    tok = ln_p.tile([128, C], BF, tag="tok")
                    if act_ln_apply:
                        nc.scalar.activation(tok[:], ps_tok[:], AF.Identity,
                                             bias=nmr[:], scale=rstd[:])
                    else:
                        nc.vector.tensor_scalar(tok[:], ps_tok[:], rstd[:], nmr[:],
                                                op0=OP.mult, op1=OP.add)
                    for ct in range(CT):
                        ps_t = pst_p.tile([128, 128], BF, tag="ps_t")
                        nc.tensor.matmul(ps_t[:], tok[:, ts(ct, 128)], ident[:],
                                         is_transpose=True, start=True, stop=True,
                                         skip_group_check=True)
                        sink(ct, ps_t)

                # ---- K group over full sequence (transpose lags 1 tile) ----
                wk = load_w_group(C)
                if _rep == 0:
                    for xc in range(1, 4):
                        nc.sync.dma_start(x_sb[:, ts(xc, TT // 4), :],
                                          xr[:, ts(xc, TT // 4), :])
                    nc.sync.dma_start(
                        wp[:], wproj.rearrange("(kt p) c -> p kt c", p=128))
                pend = None
                for tt in range(TT):
                    ps_k = ps_p.tile([128, C], dt.float32, tag="ps_k")
                    qkv_psum(ps_k, tt, wk, C)

                    def k_sink(ct, ps_t, tt=tt):
                        dst = knT[:, ct, ts(tt, 128)]
                        if with_gamma:
                            nc.vector.tensor_scalar(
                                dst, ps_t[:], gk_t[:, ct:ct + 1], bk_t[:, ct:ct + 1],
                                op0=OP.mult, op1=OP.add)
                        else:
                            nc.any.tensor_copy(dst, ps_t[:])

                    if pend is not None:
                        ln_apply_transpose(*pend)
                    pend = (ps_k, k_sink)
                ln_apply_transpose(*pend)

                # ---- Q group over query half (tiles 0..TQ-1 after permute) --
                wq = load_w_group(0)
                pend = None
                for tq in range(TQ):
                    ps_q = ps_p.tile([128, C], dt.float32, tag="ps_k")
                    qkv_psum(ps_q, tq, wq, 0)

                    def q_sink(ct, ps_t, tq=tq):
                        dst = qnT[:, ct, ts(tq, 128)]
                        if with_gamma:
                            nc.vector.tensor_scalar(
                                dst, ps_t[:], gq_t[:, ct:ct + 1], bq_t[:, ct:ct + 1],
                                op0=OP.mult, op1=OP.add)
                        else:
                            nc.any.tensor_copy(dst, ps_t[:])

                    if pend is not None:
                        ln_apply_transpose(*pend)
                    pend = (ps_q, q_sink)
                ln_apply_transpose(*pend)

                # NOTE: without gamma-folding, Q misses the softmax 1/8 scale;
                # it is folded into the Schraudolph/exp scale below instead.

                # ---- V group over full sequence ----
                wv = load_w_group(2 * C)
                for tt in range(TT):
                    ps_v = ps_p.tile([128, C], dt.float32, tag="ps_k")
                    qkv_psum(ps_v, tt, wv, 2 * C)
                    for half8 in range(2):
                        nc.vector.tensor_copy(
                            vres[:, tt, half8 * 4:half8 * 4 + 4, :]
                            .rearrange("p pr (b c) -> p pr b c", b=2)[:, :, :, 0:64],
                            ps_v[:, ts(half8, 512)]
                            .rearrange("p (pr b c) -> p pr b c", pr=4, b=2))

            # ============ P2: attention ============
            # S scale: with_gamma folds SCALE into gq on host; otherwise the
            # exp argument is scaled here (exp scale / Schraudolph multiplier).
            sscale = 1.0 if with_gamma else SCALE
            with ExitStack() as p2:
                pt_p = p2.enter_context(tc.tile_pool(name="pt", bufs=4))
                nd_p = p2.enter_context(tc.tile_pool(name="nd", bufs=2))
                nz_p = p2.enter_context(tc.tile_pool(name="nz", bufs=2))
                ps_s = p2.enter_context(tc.tile_pool(name="ps_s", bufs=1, space="PSUM"))
                ps_o = p2.enter_context(tc.tile_pool(name="ps_o", bufs=1, space="PSUM"))

                def normalize(pair, h2, numden):
                    """oT rows for (pair,h2) from evacuated [65,NQ] num+den.
                    recip = exp(-ln(den)) on ACT; bcast GpSimd; mult DVE."""
                    b0 = h2 * 64
                    lnd = nz_p.tile([1, NQ], dt.float32, tag="lnd")
                    recip = nz_p.tile([1, NQ], dt.float32, tag="recip")
                    with nc.allow_low_precision(reason="softmax denom"):
                        nc.scalar.activation(lnd[:], numden[64:65, :], AF.Ln)
                        nc.scalar.activation(recip[:], lnd[:], AF.Exp, scale=-1.0)
                    bc = nz_p.tile([64, NQ], dt.float32, tag="bc")
                    nc.gpsimd.partition_broadcast(bc[:], recip[0:1, :])
                    # all-SBUF multiply on the (otherwise idle) GpSimd keeps
                    # the DVE queue clear for the exp stream
                    nc.gpsimd.tensor_tensor(
                        oT[b0:b0 + 64, pair, :], numden[:64, :], bc[:],
                        op=OP.mult)

                pending_norm = []
                for pair in range(CT):
                    po = [ps_o.tile([128, NQ], dt.float32, tag=f"po{h2}",
                                    name=f"po{h2}")
                          for h2 in range(2)]

                    def s_exp(kt, pair=pair):
                        psums = []
                        for h2 in range(2):
                            pss = ps_s.tile([128, NQ], dt.float32, tag=f"pss{h2}",
                                            name=f"pss{h2}")
                            psums.append(pss)
                        # adjacent h0/h64 matmuls per qc chunk: complementary
                        # PE row-groups can run concurrently
                        for qc in range(2):
                            for h2 in range(2):
                                b0 = h2 * 64
                                nc.tensor.matmul(
                                    psums[h2][:, ts(qc, 512)],
                                    knT[b0:b0 + 64, pair, ts(kt, 128)],
                                    qnT[b0:b0 + 64, pair, ts(qc, 512)],
                                    start=True, stop=True, skip_group_check=True)
                        ptiles = []
                        for h2 in range(2):
                            if (kt + h2) % 2 == 0:
                                pT = pt_p.tile([128, NQ], BF, tag=f"pT{h2}")
                                nc.scalar.activation(pT[:], psums[h2][:], AF.Exp,
                                                     scale=sscale)
                            else:
                                pTi = pt_p.tile([128, NQ], dt.int16, tag=f"pT{h2}")
                                with nc.allow_low_precision(reason="schraudolph"):
                                    nc.vector.tensor_scalar(
                                        pTi[:], psums[h2][:], SCHR_A * sscale,
                                        SCHR_B, op0=OP.mult, op1=OP.add)
                                pT = pTi.bitcast(BF)
                            ptiles.append(pT)
                        return ptiles

                    def pv(kt, ptiles, pair=pair, po=po):
                        for h2 in range(2):
                            for qc in range(2):
                                nc.tensor.matmul(
                                    po[h2][:65, ts(qc, 512)],
                                    vres[:, kt, pair, h2 * 65:h2 * 65 + 65],
                                    ptiles[h2][:, ts(qc, 512)],
                                    start=(kt == 0), stop=(kt == TT - 1),
                                    skip_group_check=True)

                    prev = None
                    for kt in range(TT):
                        cur = s_exp(kt)
                        if prev is not None:
                            pv(kt - 1, prev)
                        prev = cur
                        # previous pair's normalize runs in this pair's slack
                        if kt == 4 and pending_norm:
                            for args in pending_norm:
                                normalize(*args)
                            pending_norm = []
                    pv(TT - 1, prev)

                    # evacuate num+den to SBUF so the po banks free fast;
                    # normalize is deferred into the next pair's kt loop.
                    for h2 in range(2):
                        numden = nd_p.tile([65, NQ], dt.float32, tag=f"nd{h2}",
                                           name=f"nd{h2}")
                        nc.vector.tensor_copy(numden[:], po[h2][:65, :])
                        pending_norm.append((pair, h2, numden))
                for args in pending_norm:
                    normalize(*args)
                pending_norm = []

            # ============ P3: projection ============
            with ExitStack() as p3:
                os_p = p3.enter_context(tc.tile_pool(name="os", bufs=3))
                ps_p3 = p3.enter_context(tc.tile_pool(name="ps3", bufs=4, space="PSUM"))

                for tq in range(TQ):
                    ost = os_p.tile([128, C], dt.float32, tag="ost")
                    for oc in range(2):
                        ps = ps_p3.tile([128, 512], dt.float32, tag="ps")
                        for ct in range(CT):
                            nc.tensor.matmul(
                                ps[:], oT[:, ct, ts(tq, 128)],
                                wp[:, ct, ts(oc, 512)],
                                start=(ct == 0),
                                stop=(not with_bias and ct == CT - 1),
                                skip_group_check=True)
                        if with_bias:
                            nc.tensor.matmul(
                                ps[:], ones1[:], bproj_t[:, ts(oc, 512)],
                                start=False, stop=True, skip_group_check=True)
                        nc.any.tensor_copy(ost[:, ts(oc, 512)], ps[:])
                    nc.sync.dma_start(out[ts(tq, 128), :], ost[:])

    nc.compile()
    return nc


_NCS = {}


def _get_nc(with_bias=False, with_gamma=False, reps=1):
    key = (with_bias, with_gamma, reps)
    if key not in _NCS:
        _NCS[key] = build_nc(reps=reps, with_bias=with_bias,
                             with_gamma=with_gamma)
    return _NCS[key]


def _shard_inputs(inputs, with_gamma=False):
    x = np.asarray(inputs["x"], dtype=np.float32)
    import ml_dtypes
    bf = ml_dtypes.bfloat16
    shared = {
        "wqkv": np.asarray(inputs["W_qkv"], dtype=bf),
        "wproj": np.asarray(inputs["W_proj"], dtype=bf),
        "bqkv": np.asarray(inputs["b_qkv"], dtype=bf),
        "bproj": np.asarray(inputs["b_proj"], dtype=bf),
        "gq": np.asarray(inputs["q_gamma"], dtype=np.float32) * np.float32(SCALE),
        "bq": np.asarray(inputs["q_beta"], dtype=np.float32) * np.float32(SCALE),
        "gk": np.asarray(inputs["k_gamma"], dtype=np.float32),
        "bk": np.asarray(inputs["k_beta"], dtype=np.float32),
    }
    in_maps = []
    for core in range(8):
        b, half = core // 2, core % 2
        # xTt[tt, p, kt*128+j] = x[b].T[kt*128+p, tt*128+j], with token tiles
        # permuted so this core's query half comes first.
        xt4 = x[b].T.reshape(CT, 128, TT, 128)
        xtt = np.ascontiguousarray(xt4.transpose(2, 1, 0, 3).reshape(TT, 128, C))
        if half == 1:
            xtt = np.concatenate([xtt[TQ:], xtt[:TQ]], axis=0)
        m = dict(shared)
        m["xTt"] = np.ascontiguousarray(xtt).astype(bf)
        in_maps.append(m)
    return in_maps


def kernel(**inputs) -> np.ndarray:
    from concourse.bass_utils import run_bass_kernel_spmd
    zero_bias = (not np.any(np.asarray(inputs["b_qkv"]))
                 and not np.any(np.asarray(inputs["b_proj"])))
    g1b0 = (np.allclose(np.asarray(inputs["q_gamma"]), 1.0)
            and np.allclose(np.asarray(inputs["k_gamma"]), 1.0)
            and not np.any(np.asarray(inputs["q_beta"]))
            and not np.any(np.asarray(inputs["k_beta"])))
    nc = _get_nc(with_bias=not zero_bias, with_gamma=not g1b0)
    in_maps = _shard_inputs(inputs, with_gamma=not g1b0)
    res = run_bass_kernel_spmd(nc, in_maps, core_ids=list(range(8)))
    out = np.empty((B, N, C), dtype=np.float32)
    for core in range(8):
        b, half = core // 2, core % 2
        out[b, half * NQ:(half + 1) * NQ, :] = res.results[core]["out"]
    return out
